# revision 10
# baseline (speedup 1.0000x reference)
"""nn_GAT_HE on 8 Trainium2 NeuronCores (Bass/Tile).

3 single-head GATConvs with edge embeddings, mean over heads.
N=100000 nodes, E=500000 edges, D=128, H=3, VOCAB=22754.

Strategy (dst-sharded, zero collectives):
  * Sort edges by dst; 128-node dst blocks; blocks dealt to 8 cores
    balanced by tile-class so every core compiles one identical program.
  * Weights folded on host: WA = W_lin @ [W1|W2|W3|vsrc1..3] (W part
    pre-scaled by 1/H), Vdst = W_lin @ (W_h @ att_dst_h).
  * Per 128-edge tile on device:
      y   = gather x_bf16[src]                  (indirect DMA, int32)
      yT  = PE transpose
      u   = y @ WA                              -> [e, 387] PSUM
      p   = exp(leakyrelu(a_src + a_dst))       a_dst gathered from A2 table
      S   = one-hot(dst_rel)                    (iota + is_equal)
      wp  = [p_h * ew * u_h | p]                (ew gathered from emb)
      q  += S^T @ wp                            per-block PSUM accumulation
  * Per block: r = 1/(s+eps); out = sum_h r_h * q_h + bias_mean -> arena
  * One indirect scatter writes each core's rows; host stitches cores.
"""
import numpy as np

N_NODES = 100000
N_EDGES = 500000
D = 128
H = 3
VOCAB = 22754
NEG = 0.2
P = 128
NCORES = 8
NBLK_TOT = (N_NODES + P - 1) // P
CHUNK_T = 64          # tiles per gather chunk
DUMMY_DSTREL = 300.0


def _prep(x, edge_index, edge_weight, W_lin, emb_table, W_head, att_src, att_dst, bias):
    x = np.asarray(x, np.float32)
    src = np.asarray(edge_index[0], np.int64)
    dst = np.asarray(edge_index[1], np.int64)
    ewi = np.asarray(edge_weight, np.int64)
    W_lin = np.asarray(W_lin, np.float32)
    emb = np.asarray(emb_table, np.float32)
    W_head = np.asarray(W_head, np.float32)
    att_src = np.asarray(att_src, np.float32)
    att_dst = np.asarray(att_dst, np.float32)
    bias = np.asarray(bias, np.float32)

    Wh_f = [W_lin @ W_head[h] for h in range(H)]
    vsrc = np.stack([W_lin @ (W_head[h] @ att_src[h]) for h in range(H)], 1)
    vdst = np.stack([W_lin @ (W_head[h] @ att_dst[h]) for h in range(H)], 1)
    WA = np.concatenate(Wh_f + [vsrc], axis=1)      # [D, 387]
    WA[:, :H * D] *= 1.0 / H
    bias_mean = bias.mean(0)

    order = np.argsort(dst, kind="stable")
    src_s, dst_s, ewi_s = src[order], dst[order], ewi[order]
    blk_of_edge = dst_s // P
    blk_counts = np.bincount(blk_of_edge, minlength=NBLK_TOT)
    blk_start = np.zeros(NBLK_TOT + 1, np.int64)
    np.cumsum(blk_counts, out=blk_start[1:])
    blk_tiles = np.maximum((blk_counts + P - 1) // P, 1)

    classes = sorted(set(blk_tiles.tolist()), reverse=True)
    core_blocks = [[] for _ in range(NCORES)]
    for m in classes:
        for i, b in enumerate(np.where(blk_tiles == m)[0]):
            core_blocks[i % NCORES].append(int(b))
    cls_count = {m: max(sum(1 for b in cb if blk_tiles[b] == m) for cb in core_blocks)
                 for m in classes}
    sched = []
    for m in classes:
        sched += [m] * cls_count[m]
    T_TILES = sum(sched)
    NBLK = len(sched)

    per_core = []
    for c in range(NCORES):
        cb = core_blocks[c]
        by_m = {m: [b for b in cb if blk_tiles[b] == m] for m in classes}
        seq = []
        for m in classes:
            lst = by_m[m] + [-1] * (cls_count[m] - len(by_m[m]))
            seq += lst
        E_pad = T_TILES * P
        e_src = np.zeros(E_pad, np.int32)
        e_ewi = np.zeros(E_pad, np.int32)
        e_dstl = np.zeros(E_pad, np.int32)
        e_dstrel = np.full(E_pad, DUMMY_DSTREL, np.float32)
        xT = np.zeros((D, NBLK * P), np.float32)
        scat = np.empty((P, NBLK), np.int32)
        pos = 0
        for bi, (m, b) in enumerate(zip(sched, seq)):
            scat[:, bi] = bi * P + np.arange(P)
            if b >= 0:
                s, e = blk_start[b], blk_start[b + 1]
                n = int(e - s)
                e_src[pos:pos + n] = src_s[s:e]
                e_ewi[pos:pos + n] = ewi_s[s:e]
                e_dstrel[pos:pos + n] = (dst_s[s:e] - b * P).astype(np.float32)
                e_dstl[pos:pos + n] = bi * P + (dst_s[s:e] - b * P)
                rows = np.arange(b * P, min((b + 1) * P, N_NODES))
                xT[:, bi * P: bi * P + len(rows)] = x[rows].T
            pos += m * P
        per_core.append(dict(
            seq=seq,
            e_src=np.ascontiguousarray(e_src.reshape(T_TILES, P).T),
            e_ewi=np.ascontiguousarray(e_ewi.reshape(T_TILES, P).T),
            e_dstl=np.ascontiguousarray(e_dstl.reshape(T_TILES, P).T),
            e_dstrel=np.ascontiguousarray(e_dstrel.reshape(T_TILES, P).T),
            xT=xT, scat=scat))

    struct = dict(sched=sched, T_TILES=T_TILES, NBLK=NBLK)
    consts = dict(WA=WA, vdst=vdst, bias_mean=bias_mean, emb=emb, x=x)
    return per_core, struct, consts


_TILE_PATCHED = False


def _patch_tile_drain():
    """This walrus build accepts only one sync-wait per Drain; TileContext's
    tail drain attaches the whole global clock. Split waits across drains."""
    global _TILE_PATCHED
    if _TILE_PATCHED:
        return
    _TILE_PATCHED = True
    import bass_rust
    import concourse.tile as tile
    from concourse.vector_clock import ScopedClock

    def _drain_and_barrier(self, tick_clock, wait_clock):
        drain_inst = self.nc.sync.drain()
        wait_clock.add_sem_waits(
            drain_inst.ins, ScopedClock({None: tick_clock.global_clock}))
        si = drain_inst.ins.sync_info
        waits = list(si.on_wait) if si is not None and si.on_wait else []
        upd = list(si.on_update) if si is not None and si.on_update else []
        if len(waits) > 1:
            drain_inst.ins.sync_info = bass_rust.SyncInfo(
                on_wait=waits[:1], on_update=[])
            for k, w in enumerate(waits[1:]):
                d2 = self.nc.sync.drain()
                is_last = k == len(waits) - 2
                d2.ins.sync_info = bass_rust.SyncInfo(
                    on_wait=[w], on_update=upd if is_last else [])
        self.nc.all_engine_barrier()
        popped = self.nc._tile_sem_poison_stack.pop()
        assert popped is self._sem_poison
        self.nc.clear_and_free_semaphores(list(self.sems.allocated().values()))
        self.nc.all_engine_barrier()

    tile.TileContext._drain_and_barrier = _drain_and_barrier

    # Walrus also rejects >1 sync-wait on regular instructions. Before the
    # ordered lists are committed into basic blocks, spill extra waits onto
    # freshly created same-engine nops inserted just before the instruction.
    orig_lower = tile.TileContext._lower_ordered_insts

    def _lower_with_wait_split(self, ordered):
        nc = self.nc
        for bb_name, insts in ordered.items():
            out = []
            for inst in insts:
                si = inst.sync_info
                waits = list(si.on_wait) if si is not None and si.on_wait else []
                if len(waits) > 1 and inst.is_executable():
                    upd = list(si.on_update) if si.on_update else []
                    for w in waits[:-1]:
                        nop = nc.engines[inst.engine].nop()
                        nop.ins.sync_info = bass_rust.SyncInfo(
                            on_wait=[w], on_update=[])
                        out.append(nop.ins)
                    inst.sync_info = bass_rust.SyncInfo(
                        on_wait=waits[-1:], on_update=upd)
                out.append(inst)
            ordered[bb_name] = out
        return orig_lower(self, ordered)

    tile.TileContext._lower_ordered_insts = _lower_with_wait_split


def _build(struct):
    import concourse.bass as bass
    import concourse.mybir as mybir
    import concourse.tile as tile
    from concourse.masks import make_identity

    _patch_tile_drain()

    sched = struct["sched"]
    T = struct["T_TILES"]
    NBLK = struct["NBLK"]
    f32, bf16, i32 = mybir.dt.float32, mybir.dt.bfloat16, mybir.dt.int32
    Alu = mybir.AluOpType
    Act = mybir.ActivationFunctionType

    nc = bass.Bass()
    x_hbm = nc.declare_dram_parameter("x", [N_NODES, D], bf16, isOutput=False)
    emb_hbm = nc.declare_dram_parameter("emb", [VOCAB, D], bf16, isOutput=False)
    WA_hbm = nc.declare_dram_parameter("WA", [D, 387], bf16, isOutput=False)
    vdst_hbm = nc.declare_dram_parameter("vdst", [D, 3], bf16, isOutput=False)
    iota_hbm = nc.declare_dram_parameter("iota", [P, P], bf16, isOutput=False)
    biasB_hbm = nc.declare_dram_parameter("biasB", [P, D], f32, isOutput=False)
    xT_hbm = nc.declare_dram_parameter("xT", [D, NBLK * P], bf16, isOutput=False)
    src_hbm = nc.declare_dram_parameter("esrc", [P, T], i32, isOutput=False)
    ewi_hbm = nc.declare_dram_parameter("eewi", [P, T], i32, isOutput=False)
    dstrel_hbm = nc.declare_dram_parameter("edstrel", [P, T], f32, isOutput=False)
    out_hbm = nc.declare_dram_parameter("out_loc", [NBLK * P + P, D], f32, isOutput=True)


    with tile.TileContext(nc) as tc:
        with (
            tc.tile_pool(name="const", bufs=1) as cpool,
            tc.tile_pool(name="xtp", bufs=3) as xtp,
            tc.tile_pool(name="chunk", bufs=2) as chp,
            tc.tile_pool(name="work", bufs=3) as wk,
            tc.tile_pool(name="flush", bufs=2) as fl,
            tc.tile_pool(name="arena", bufs=1) as arp,
            tc.tile_pool(name="pt", bufs=2, space="PSUM") as pt,
            tc.tile_pool(name="pu", bufs=2, space="PSUM") as pu,
            tc.tile_pool(name="pq", bufs=2, space="PSUM") as pq,
        ):
            ident = cpool.tile([P, P], bf16)
            make_identity(nc, ident[:])
            WA_sb = cpool.tile([D, 387], bf16)
            nc.sync.dma_start(out=WA_sb[:], in_=WA_hbm[:])
            vdst_sb = cpool.tile([D, 3], bf16)
            nc.sync.dma_start(out=vdst_sb[:], in_=vdst_hbm[:])
            iota_sb = cpool.tile([P, P], bf16)
            nc.sync.dma_start(out=iota_sb[:], in_=iota_hbm[:])
            biasB_sb = cpool.tile([P, D], f32)
            nc.sync.dma_start(out=biasB_sb[:], in_=biasB_hbm[:])
            dstrel_sb = cpool.tile([P, T], f32)
            nc.sync.dma_start(out=dstrel_sb[:], in_=dstrel_hbm[:])
            a2_sb = cpool.tile([P, NBLK, 8], bf16)
            arena = arp.tile([P, NBLK * D], f32)

            # ---- phase 1: A2 table (a_dst per node), kept in SBUF ----
            for bi in range(NBLK):
                xts = xtp.tile([D, P], bf16, tag="xts")
                nc.sync.dma_start(out=xts[:], in_=xT_hbm[:, bi * P:(bi + 1) * P])
                a2_ps = pu.tile([P, 390], f32, tag="ups")
                nc.tensor.matmul(out=a2_ps[:, 0:3], lhsT=xts[:], rhs=vdst_sb[:],
                                 start=True, stop=True)
                nc.vector.tensor_copy(out=a2_sb[:, bi, 0:3], in_=a2_ps[:, 0:3])

            # ---- phase 2: main edge loop ----
            tile_of = []           # (block_index, first, last) per tile
            for bi, m in enumerate(sched):
                for t in range(m):
                    tile_of.append((bi, t == 0, t == m - 1))
            idx_sb = cpool.tile([P, T], i32, tag="srcidx")
            nc.sync.dma_start(out=idx_sb[:], in_=src_hbm[:])
            ewi_sb = cpool.tile([P, T], i32, tag="ewiidx")
            nc.sync.dma_start(out=ewi_sb[:], in_=ewi_hbm[:])

            q_ps = None
            for t in range(T):
                bi, first, last = tile_of[t]

                ysl = wk.tile([P, D], bf16, tag="ytile")
                nc.gpsimd.indirect_dma_start(
                    out=ysl[:], out_offset=None, in_=x_hbm[:],
                    in_offset=bass.IndirectOffsetOnAxis(
                        ap=idx_sb[:, t:t + 1], axis=0))
                ewsl = wk.tile([P, D], bf16, tag="ewtile")
                nc.gpsimd.indirect_dma_start(
                    out=ewsl[:], out_offset=None, in_=emb_hbm[:],
                    in_offset=bass.IndirectOffsetOnAxis(
                        ap=ewi_sb[:, t:t + 1], axis=0))

                S_sb = wk.tile([P, P], bf16, tag="ssb")
                nc.vector.tensor_scalar(out=S_sb[:], in0=iota_sb[:],
                                        scalar1=dstrel_sb[:, t:t + 1],
                                        scalar2=None, op0=Alu.is_equal)

                yT_ps = pt.tile([P, P], bf16, tag="ytps")
                nc.tensor.transpose(out=yT_ps[:], in_=ysl[:], identity=ident[:])
                ST_ps = pt.tile([P, P], bf16, tag="stps")
                nc.tensor.transpose(out=ST_ps[:], in_=S_sb[:], identity=ident[:])
                yT = wk.tile([P, P], bf16, tag="ytsb")
                nc.scalar.activation(out=yT[:], in_=yT_ps[:], func=Act.Copy)
                ST = wk.tile([P, P], bf16, tag="stsb")
                nc.vector.tensor_copy(out=ST[:], in_=ST_ps[:])

                u_ps = pu.tile([P, 390], f32, tag="ups")
                nc.tensor.matmul(out=u_ps[:, 0:387], lhsT=yT[:], rhs=WA_sb[:],
                                 start=True, stop=True)
                nc.tensor.matmul(out=u_ps[:, 387:390], lhsT=ST[:],
                                 rhs=a2_sb[:, bi, 0:3],
                                 start=True, stop=True, skip_group_check=True)
                ad_sb = wk.tile([P, 3], f32, tag="adsb")
                nc.vector.tensor_copy(out=ad_sb[:], in_=u_ps[:, 387:390])

                wp = wk.tile([P, 387], bf16, tag="wp")
                zt = wk.tile([P, 3], f32, tag="zt")
                nc.vector.tensor_tensor(out=zt[:], in0=u_ps[:, 384:387],
                                        in1=ad_sb[:], op=Alu.add)
                zl = wk.tile([P, 3], f32, tag="zl")
                nc.scalar.activation(out=zl[:], in_=zt[:], func=Act.Prelu, alpha=NEG)
                p32 = wk.tile([P, 3], f32, tag="p32")
                nc.scalar.activation(out=p32[:], in_=zl[:], func=Act.Exp)
                nc.scalar.activation(out=wp[:, 384:387], in_=zl[:], func=Act.Exp)

                for h in range(H):
                    hsl = slice(h * D, (h + 1) * D)
                    nc.vector.tensor_tensor(out=wp[:, hsl], in0=u_ps[:, hsl],
                                            in1=ewsl[:], op=Alu.mult)
                for h, eng in zip(range(H), ("act", "act", "dve")):
                    hsl = slice(h * D, (h + 1) * D)
                    if eng == "act":
                        nc.scalar.activation(out=wp[:, hsl], in_=wp[:, hsl],
                                             func=Act.Copy,
                                             scale=p32[:, h:h + 1])
                    else:
                        nc.vector.tensor_scalar(out=wp[:, hsl], in0=wp[:, hsl],
                                                scalar1=p32[:, h:h + 1],
                                                scalar2=None, op0=Alu.mult)

                if first:
                    q_ps = pq.tile([P, 387], f32, tag="qps")
                nc.tensor.matmul(out=q_ps[:], lhsT=S_sb[:], rhs=wp[:],
                                 start=first, stop=last)

                if last:
                    s3 = fl.tile([P, 3], f32, tag="s3")
                    nc.vector.tensor_scalar(out=s3[:], in0=q_ps[:, 384:387],
                                            scalar1=1e-16, scalar2=None,
                                            op0=Alu.add)
                    r3 = fl.tile([P, 3], f32, tag="r3")
                    nc.vector.reciprocal(out=r3[:], in_=s3[:])
                    t1 = fl.tile([P, D], f32, tag="t1")
                    t2 = fl.tile([P, D], f32, tag="t2")
                    nc.vector.tensor_scalar(out=t1[:], in0=q_ps[:, 0:D],
                                            scalar1=r3[:, 0:1], scalar2=None,
                                            op0=Alu.mult)
                    nc.vector.tensor_scalar(out=t2[:], in0=q_ps[:, D:2 * D],
                                            scalar1=r3[:, 1:2], scalar2=None,
                                            op0=Alu.mult)
                    nc.vector.tensor_tensor(out=t1[:], in0=t1[:], in1=t2[:],
                                            op=Alu.add)
                    nc.vector.tensor_scalar(out=t2[:], in0=q_ps[:, 2 * D:3 * D],
                                            scalar1=r3[:, 2:3], scalar2=None,
                                            op0=Alu.mult)
                    nc.vector.tensor_tensor(out=t1[:], in0=t1[:], in1=t2[:],
                                            op=Alu.add)
                    nc.vector.tensor_tensor(out=arena[:, bi * D:(bi + 1) * D],
                                            in0=t1[:], in1=biasB_sb[:],
                                            op=Alu.add)

            # ---- phase 3: arena -> out rows (identity layout, direct DMA) ----
            nc.sync.dma_start(
                out=out_hbm[:NBLK * P].rearrange("(b p) d -> p b d", p=P),
                in_=arena[:].rearrange("p (b d) -> p b d", d=D))

    return nc


def _make_in_maps(per_core, consts):
    import ml_dtypes
    bf16 = ml_dtypes.bfloat16
    x_bf = consts["x"].astype(bf16)
    emb_bf = consts["emb"].astype(bf16)
    WA_bf = consts["WA"].astype(bf16)
    vdst_bf = consts["vdst"].astype(bf16)
    iota = np.ascontiguousarray(
        np.broadcast_to(np.arange(P, dtype=np.float32), (P, P)).astype(bf16))
    biasB = np.ascontiguousarray(
        np.broadcast_to(consts["bias_mean"], (P, D)).astype(np.float32))
    in_maps = []
    for pc in per_core:
        in_maps.append({
            "x": x_bf, "emb": emb_bf, "WA": WA_bf, "vdst": vdst_bf,
            "iota": iota, "biasB": biasB,
            "xT": pc["xT"].astype(bf16),
            "esrc": pc["e_src"], "eewi": pc["e_ewi"],
            "edstrel": pc["e_dstrel"],
        })
    return in_maps


def _combine(per_core, results):
    full = np.zeros((N_NODES, D), np.float32)
    for c in range(NCORES):
        o = np.asarray(results[c]["out_loc"], np.float32)
        for bi, b in enumerate(per_core[c]["seq"]):
            if b < 0:
                continue
            lo = b * P
            hi = min(lo + P, N_NODES)
            full[lo:hi] = o[bi * P: bi * P + (hi - lo)]
    return full


def kernel(x, edge_index, edge_weight, W_lin, emb_table, W_head, att_src, att_dst, bias):
    from concourse.bass_utils import run_bass_kernel_spmd

    per_core, struct, consts = _prep(
        x, edge_index, edge_weight, W_lin, emb_table, W_head, att_src, att_dst, bias)
    nc = _build(struct)
    in_maps = _make_in_maps(per_core, consts)
    res = run_bass_kernel_spmd(nc, in_maps, list(range(NCORES)))
    return _combine(per_core, [res.results[c] for c in range(NCORES)])
